# revision 32
# baseline (speedup 1.0000x reference)
"""NT-Xent loss kernel for Trainium2, 8-core SPMD, collective-free.

Math: with p = cat(z_i, z_j) [8192, 64], pn = p / max(||p||, 1e-8),
sim = 2 * pn @ pn.T (TEMP=0.5), the reference's gather-based losses reduce to
  loss1 = mean_r( log(sum_c exp(sim[r,c]) - exp(sim[r,r])) - pos_r )
  loss2 = mean_r( log(exp(pos_r) + sum_{c != t_r} exp(probs[r,c])) - pos_r )
where pos_r = sim[r, partner(r)].  sim entries lie in [-2.6, 2.6] so no
max-shift is needed.  The huge neg_idx input is a fixed structured mask and
is never read; the probs negative selection (drop own class) is 8192x10
index prep done on host.

Both losses are sums of per-row terms, each depending only on the full
column set, so any symmetric permutation of rows leaves them unchanged.  We
permute rows so core k owns rows [z_i[k*512:(k+1)*512]; z_j[k*512:...]]:
every row's positive partner lives at +-512 rows on the same core.

v2: the v1 on-device AllGather of normalized rows measured 47-77us of
mostly sync latency -- the dominant cost.  It is eliminated by normalizing
+ transposing on host and shipping the full pnT [64, 8192] REPLICATED to
every core as fp8e4m3 (512KB/core; the fp8 quantization perturbs the final
losses by ~1e-5 measured against the f64 reference, since independent
per-element errors cancel in the 8k-term row sums).  The PE reads fp8
operands at bf16 speed, so the sim matmuls need no upcast; each core's
stationary lhsT slice ships separately ([64,1024] fp8) because ldweights
cannot take register offsets.  pos/diag row dots ship precomputed in f32
from the SAME fp8 values the matmul sees (consistency), exp runs in
2048-wide activates (amortizing the ~352-cycle ACT pipeline fill) writing
bf16, and the row sums run on DVE at the 2x 16-bit rate.  Zero cross-core
traffic; host sums the 8 partial loss pairs.

Dispatch: run_bass_kernel_spmd rebuilds its jax.jit on every call; the
first kernel() call runs through it, then builds a cached-jit runner around
the same _bass_exec_p primitive, verifies it, and later calls use that.
"""

import numpy as np
import ml_dtypes

import concourse.bass as bass
import concourse.bacc as bacc
import concourse.tile as tile
from concourse import mybir
from concourse.bass_utils import run_bass_kernel_spmd

N = 4096
D = 64
M = 2 * N            # 8192 rows of sim
NCORES = 8
R = M // NCORES      # 1024 rows per core
NS = R // 128        # 8 row-chunks of 128 per core
NG = 4               # column groups of 2048 in the main loop
GW = M // NG         # 2048 columns per group
NCLS = 10
NNEG = NCLS - 1      # 9 probs negatives per row
BC = NNEG + 2        # blob cols: 9 probs negs + diag + pos (bf16)
INV_TEMP = 2.0       # 1 / 0.5
F32 = mybir.dt.float32
BF16 = mybir.dt.bfloat16
FP8 = mybir.dt.float8e4

AF = mybir.ActivationFunctionType
ALU = mybir.AluOpType


def build_program():
    nc = bacc.Bacc("TRN2", target_bir_lowering=False, debug=False,
                   num_devices=NCORES)

    own_d = nc.dram_tensor("own8", [D, R], FP8, kind="ExternalInput").ap()
    pnt_d = nc.dram_tensor("pnt8", [D, M], FP8, kind="ExternalInput").ap()
    blob_d = nc.dram_tensor("blob", [R, BC], BF16,
                            kind="ExternalInput").ap()
    out_d = nc.dram_tensor("out", [1, 2], F32, kind="ExternalOutput").ap()

    with tile.TileContext(nc) as tc:
        with tc.tile_pool(name="consts", bufs=1) as consts, \
             tc.tile_pool(name="big", bufs=1) as big, \
             tc.tile_pool(name="mm", bufs=2, space="PSUM") as mm, \
             tc.tile_pool(name="esc", bufs=4) as esc, \
             tc.tile_pool(name="acc", bufs=3) as acc:

            # ---- load inputs ---------------------------------------------
            # pnt in two halves so the first matmuls start after half the
            # transfer (the main loop's g=0,1 only touch the low half), and
            # the four issues spread over four idle engine queues so they
            # dispatch in parallel instead of serializing on sync
            own = big.tile([D, R], FP8)
            nc.scalar.dma_start(out=own, in_=own_d)
            pnt = big.tile([D, M], FP8)
            nc.sync.dma_start(out=pnt[:, 0:512], in_=pnt_d[:, 0:512])
            nc.gpsimd.dma_start(out=pnt[:, 512:GW], in_=pnt_d[:, 512:GW])
            nc.sync.dma_start(out=pnt[:, GW:2 * GW], in_=pnt_d[:, GW:2 * GW])
            nc.sync.dma_start(out=pnt[:, 2 * GW:M], in_=pnt_d[:, 2 * GW:M])
            braw = big.tile([128, NS, BC], BF16)
            nc.sync.dma_start(
                out=braw, in_=blob_d.rearrange("(n p) c -> p n c", p=128))

            ones = consts.tile([128, 1], F32)
            nc.vector.memset(ones, 1.0)

            # ---- pos/diag + probs prep: runs in the ACT idle window
            # between the exp table load and the first main-loop exp -------
            diag_raw = big.tile([128, NS], F32)
            nc.vector.tensor_copy(diag_raw, braw[:, :, NNEG])
            pos_raw = big.tile([128, NS], F32)
            nc.vector.tensor_copy(pos_raw, braw[:, :, NNEG + 1])
            ediag = big.tile([128, NS], F32)
            nc.scalar.activation(ediag, diag_raw, AF.Exp, scale=INV_TEMP)
            epos = big.tile([128, NS], F32)
            nc.scalar.activation(epos, pos_raw, AF.Exp, scale=INV_TEMP)
            pos2 = big.tile([128, NS], F32)
            nc.vector.tensor_scalar_mul(pos2, pos_raw, INV_TEMP)

            eprobs = big.tile([128, NS, NNEG], F32)
            nc.scalar.activation(eprobs, braw[:, :, 0:NNEG], AF.Exp)
            ps2 = big.tile([128, NS], F32)
            nc.vector.tensor_reduce(ps2, eprobs, axis=mybir.AxisListType.X,
                                    op=ALU.add)

            # ---- main loop: exp(sim slab) row sums ------------------------
            # row sums per 128-row chunk: DVE pair-add tree over the 4 bf16
            # exp tiles (tensor_tensor runs 2x on bf16, tensor_reduce only
            # 1x), except the last chunk which accumulates on ACT so the
            # final row sums don't trail the last exp by a reduce chain.
            stot = big.tile([128, NS], F32)
            sacc = big.tile([128, NG], F32)
            for n in range(NS):
                lhsT = own[:, n * 128:(n + 1) * 128]
                last = n == NS - 1
                ets = []
                for g in range(NG):
                    pst = mm.tile([128, GW], F32, tag="mm")
                    for q in range(4):
                        c0 = g * GW + q * 512
                        nc.tensor.matmul(pst[:, q * 512:(q + 1) * 512], lhsT,
                                         pnt[:, c0:c0 + 512],
                                         start=True, stop=True)
                    et = esc.tile([128, GW], BF16, tag="esc")
                    if last:
                        nc.scalar.activation(et, pst, AF.Exp, scale=INV_TEMP,
                                             accum_out=sacc[:, g:g + 1])
                    else:
                        nc.scalar.activation(et, pst, AF.Exp, scale=INV_TEMP)
                        ets.append(et)
                if last:
                    nc.vector.tensor_reduce(stot[:, n:n + 1], sacc,
                                            axis=mybir.AxisListType.X,
                                            op=ALU.add)
                else:
                    a01 = acc.tile([128, GW], BF16, tag="acc")
                    nc.vector.tensor_add(a01, ets[0], ets[1])
                    a23 = acc.tile([128, GW], BF16, tag="acc")
                    nc.vector.tensor_add(a23, ets[2], ets[3])
                    a03 = acc.tile([128, GW], BF16, tag="acc")
                    nc.vector.tensor_add(a03, a01, a23)
                    nc.vector.tensor_reduce(stot[:, n:n + 1], a03,
                                            axis=mybir.AxisListType.X,
                                            op=ALU.add)

            # ---- loss tails -----------------------------------------------
            s1 = big.tile([128, NS], F32)
            nc.vector.tensor_sub(s1, stot, ediag)
            lse1 = big.tile([128, NS], F32)
            nc.scalar.activation(lse1, s1, AF.Ln)
            c1 = big.tile([128, NS], F32)
            nc.vector.tensor_sub(c1, lse1, pos2)
            v12 = big.tile([128, 2], F32)
            nc.vector.tensor_reduce(v12[:, 0:1], c1,
                                    axis=mybir.AxisListType.X, op=ALU.add)

            s2 = big.tile([128, NS], F32)
            nc.vector.tensor_add(s2, ps2, epos)
            # false data-dep on stot so the scheduler cannot hoist the Ln
            # into the exp stream (each hoist costs 2 ACT table swaps)
            nc.vector.scalar_tensor_tensor(
                out=s2, in0=stot, scalar=0.0, in1=s2,
                op0=ALU.mult, op1=ALU.add)
            lse2 = big.tile([128, NS], F32)
            nc.scalar.activation(lse2, s2, AF.Ln)
            c2 = big.tile([128, NS], F32)
            nc.vector.tensor_sub(c2, lse2, pos2)
            nc.vector.tensor_reduce(v12[:, 1:2], c2,
                                    axis=mybir.AxisListType.X, op=ALU.add)

            # ---- partition-sum via ones-matmul, then DMA out --------------
            pso = mm.tile([128, GW], F32, tag="mm")
            nc.tensor.matmul(pso[0:1, 0:2], ones, v12, start=True, stop=True)
            outsb = big.tile([1, 2], F32)
            nc.vector.tensor_copy(outsb, pso[0:1, 0:2])
            nc.sync.dma_start(out=out_d, in_=outsb)

    nc.compile()
    return nc


_NC_CACHE = None


def _get_nc():
    global _NC_CACHE
    if _NC_CACHE is None:
        _NC_CACHE = build_program()
    return _NC_CACHE


def make_in_maps(z_i, z_j, probs, target):
    """pnt8: [64, 8192] fp8 pn.T in permuted row order (replicated);
    own8: this core's [64, 1024] slice of pnt8 (the stationary operand);
    blob: per-core [1024, 11] bf16 -- 9 probs negatives (own class
    dropped), then the diag and pos dots of the fp8 values."""
    z_i = np.asarray(z_i, np.float32)
    z_j = np.asarray(z_j, np.float32)
    probs = np.asarray(probs, np.float32)
    t2 = np.concatenate([np.asarray(target), np.asarray(target)])
    keep = np.arange(NCLS)[None, :] != t2[:, None]
    pn9 = probs[keep].reshape(M, NNEG)

    half = R // 2
    p = np.empty((NCORES, R, D), np.float32)
    p[:, :half, :] = z_i.reshape(NCORES, half, D)
    p[:, half:, :] = z_j.reshape(NCORES, half, D)
    p = p.reshape(M, D)
    pn = p / np.maximum(np.sqrt((p * p).sum(1, keepdims=True)), 1e-8)
    q8 = pn.astype(ml_dtypes.float8_e4m3)
    q = q8.astype(np.float32)
    pnt8 = np.ascontiguousarray(q8.T)

    qc = q.reshape(NCORES, R, D)
    diag = np.einsum('krd,krd->kr', qc, qc)
    posv = np.einsum('krd,krd->kr', qc,
                     np.concatenate([qc[:, half:], qc[:, :half]], axis=1))

    blob = np.empty((NCORES, R, BC), ml_dtypes.bfloat16)
    blob[:, :half, 0:NNEG] = pn9[:N].reshape(NCORES, half, NNEG)
    blob[:, half:, 0:NNEG] = pn9[N:].reshape(NCORES, half, NNEG)
    blob[:, :, NNEG] = diag
    blob[:, :, NNEG + 1] = posv

    return [{"pnt8": pnt8,
             "own8": np.ascontiguousarray(pnt8[:, k * R:(k + 1) * R]),
             "blob": blob[k]} for k in range(NCORES)]


def _assemble(results):
    parts = np.stack([results[k]["out"].reshape(2) for k in range(NCORES)])
    total = parts.sum(axis=0) / np.float32(M)
    return (np.asarray(np.float32(total[0])), np.asarray(np.float32(total[1])))


class _CachedRunner:
    """run_bass_via_pjrt with the jitted executable built once and the
    donated output zeros created on device."""

    def __init__(self, nc, n_cores):
        import jax
        import jax.numpy as jnp
        from jax.sharding import Mesh, PartitionSpec, NamedSharding
        import warnings
        with warnings.catch_warnings():
            warnings.simplefilter("ignore")
            from jax.experimental.shard_map import shard_map
        from concourse import bass2jax

        bass2jax.install_neuronx_cc_hook()
        self._jax = jax
        self._np = np
        partition_name = (nc.partition_id_tensor.name
                          if nc.partition_id_tensor else None)

        in_names, out_names, out_avals, zero_shapes = [], [], [], []
        for alloc in nc.m.functions[0].allocations:
            if not isinstance(alloc, mybir.MemoryLocationSet):
                continue
            name = alloc.memorylocations[0].name
            if alloc.kind == "ExternalInput":
                if name != partition_name:
                    in_names.append(name)
            elif alloc.kind == "ExternalOutput":
                out_names.append(name)
                shape = tuple(alloc.tensor_shape)
                dtype = mybir.dt.np(alloc.dtype)
                out_avals.append(jax.core.ShapedArray(shape, dtype))
                zero_shapes.append((shape, dtype))
        n_params = len(in_names)
        n_outs = len(out_avals)
        all_in_names = list(in_names) + list(out_names)
        if partition_name is not None:
            all_in_names.append(partition_name)
        donate = tuple(range(n_params, n_params + n_outs))
        self._in_names = in_names
        self._out_names = out_names
        self._out_avals = out_avals
        self._n_cores = n_cores

        def _body(*args):
            operands = list(args)
            if partition_name is not None:
                operands.append(bass2jax.partition_id_tensor())
            outs = bass2jax._bass_exec_p.bind(
                *operands,
                out_avals=tuple(out_avals),
                in_names=tuple(all_in_names),
                out_names=tuple(out_names),
                lowering_input_output_aliases=(),
                sim_require_finite=True,
                sim_require_nnan=True,
                nc=nc,
            )
            return tuple(outs)

        devices = jax.devices()[:n_cores]
        mesh = Mesh(np.asarray(devices), ("core",))
        in_specs = (PartitionSpec("core"),) * (n_params + n_outs)
        out_specs = (PartitionSpec("core"),) * len(out_names)
        self._sharded = jax.jit(
            shard_map(_body, mesh=mesh, in_specs=in_specs,
                      out_specs=out_specs, check_rep=False),
            donate_argnums=donate, keep_unused=True,
        )
        csh = NamedSharding(mesh, PartitionSpec("core"))

        def _zeros():
            return tuple(
                jnp.zeros((n_cores * s[0], *s[1:]), d)
                for s, d in zero_shapes)

        self._zf = jax.jit(_zeros, out_shardings=(csh,) * n_outs)

    def run(self, in_maps):
        np_ = self._np
        per_core = [[np_.asarray(m[name]) for name in self._in_names]
                    for m in in_maps]
        concat_in = [
            np_.concatenate([per_core[c][i] for c in range(self._n_cores)],
                            axis=0)
            for i in range(len(self._in_names))
        ]
        zeros = self._zf()
        out_arrs = self._sharded(*concat_in, *zeros)
        return [
            {
                name: np_.asarray(out_arrs[i]).reshape(
                    self._n_cores, *self._out_avals[i].shape)[c]
                for i, name in enumerate(self._out_names)
            }
            for c in range(self._n_cores)
        ]


_RUNNER = None
_RUNNER_FAILED = False


def kernel(z_i, z_j, probs, target, neg_idx):
    # neg_idx is the fixed structured NT-Xent mask (all columns except self
    # and positive); its effect is computed analytically, so it's never read.
    del neg_idx
    global _RUNNER, _RUNNER_FAILED
    nc = _get_nc()
    in_maps = make_in_maps(z_i, z_j, probs, target)

    if _RUNNER is not None:
        try:
            return _assemble(_RUNNER.run(in_maps))
        except Exception:
            # tunnel hiccup or runner breakage: permanently fall back to
            # the stock dispatch path
            _RUNNER = None
            _RUNNER_FAILED = True

    try:
        res = run_bass_kernel_spmd(nc, in_maps, list(range(NCORES)))
        out = _assemble(res.results)
    except Exception:
        # e.g. BASS_TRACE=1 in an env without antenv.axon_hooks
        out = None

    if not _RUNNER_FAILED:
        try:
            runner = _CachedRunner(nc, NCORES)
            chk = _assemble(runner.run(in_maps))
            if out is None:
                out = chk
                _RUNNER = runner
            elif (abs(float(chk[0]) - float(out[0]))
                    <= 1e-4 * abs(float(out[0]))
                    and abs(float(chk[1]) - float(out[1]))
                    <= 1e-4 * abs(float(out[1]))):
                _RUNNER = runner
            else:
                _RUNNER_FAILED = True
        except Exception:
            _RUNNER_FAILED = True
    if out is None:
        raise RuntimeError(
            "both the stock run_bass_kernel_spmd dispatch and the cached "
            "runner failed")
    return out
